# revision 3
# baseline (speedup 1.0000x reference)
"""Butterfly (10-stage, n=1024) on 8 TRN2 cores — 3-phase factorized kernel.

Math: with nstack=1, the 10 butterfly stages compose into a dense W
(out = W x + bias). Split position index p = 128*k + s, s = 16*g + l:
stages 0-6 form a block-diagonal C (8 independent 128x128 blocks, block k
maps c-block k -> s of block k); stages 7-9 mix ONLY k at fixed s (an 8x8
matrix H8[s] per position-within-block s).

Device pipeline per super-tile (sbt = 512 batch rows; core handles 8):
  A: 32 matmuls, x-chunks STATIONARY ([c'=128, b=128] bf16), C-blocks
     moving (N=128) -> z' batch-major [b=128, (k, s)] f32 in PSUM.
     Copy to SBUF bf16 with (k, (g, l)) -> (g, k, l) column permute
     (ACT/DVE split).
  T: 32 PE transposes of z' slices [b=128, (k,l)=128] -> zint [(k,l), b]
     bf16 in PSUM — this performs the partition interleave that any
     2-phase scheme must otherwise pay 2x PE streaming for.
     Copies to SBUF as pure bit-moves (bf16 pairs viewed as f32: half
     the elements).
  B: 8 matmuls (N=512), stationary hg[g] = H8 entries packed per 16-row
     group g; contract over all 128 (k,l) -> out tile [(k',l'), b] f32.
     Bias added during the PSUM->SBUF copy (per-partition scalar);
     output staged bf16 and written with ONE merged 1MB DMA per sbt
     (8KB contiguous per partition line).

PE cost is 12,288 moving-rows/sbt (the structural floor: 3 full passes
over the data; any transpose-free 2-phase split conserves 24,576 —
that 2x tax is exactly what the previous 2-level kernel paid). PE floor
= 98,304 rows/pass @ 2.4GHz = 41.0us. Measured (quiet device): 42.4 and
45.7us/pass vs 86.5-90.7us for the previous kernel (~2x); max rel err
vs f64 reference 6.36e-3 (gate 2e-2). bf16 in/out halves DMA vs f32;
merged 8KB-line descriptors keep DMA far off the critical path.

Per-core DRAM I/O: xt 8MB in, out 8MB out (bf16).
"""

import numpy as np
import ml_dtypes

import concourse.bass as bass
import concourse.bacc as bacc
import concourse.mybir as mybir
from concourse.tile import TileContext
from concourse.bass_utils import run_bass_kernel_spmd

N_CORES = 8
BATCH = 32768
NPOS = 1024
P = 128
SBT_PER_CORE = 8
NCHUNK = 8

# engine-balance / buffering defaults (HW-tuned)
CFG = dict(act_b=4, xtp_bufs=3, zint_bufs=3, op_bufs=3, a_act=2, t_act=3)


def _apply_stages(tw, v, stages):
    """Apply butterfly stages (f64) to rows of v: v |-> B_stages v."""
    b, n = v.shape
    out = v.reshape(b, 1, n)
    tw = np.asarray(tw, dtype=np.float64)
    for idx in stages:
        stride = 1 << idx
        nb = n // (2 * stride)
        t = tw[:, idx].reshape(1, nb, stride, 2, 2).transpose(0, 1, 3, 4, 2)
        o = out.reshape(b, 1, nb, 1, 2, stride)
        out = (t * o).sum(axis=4).reshape(b, 1, n)
    return out.reshape(b, n)


def _compose_w(twiddle):
    """Full composed transform: M[c, p] with out = x @ M (for host checks)."""
    return _apply_stages(twiddle, np.eye(NPOS), range(10))


def _pidx():
    """pidx[kl, g] = position 128*(kl//16) + 16*g + (kl%16)."""
    kl = np.arange(128)
    g = np.arange(8)
    return 128 * (kl[:, None] // 16) + 16 * g[None, :] + (kl[:, None] % 16)


def _pack_xs(x, twiddle, bias):
    x = np.asarray(x, dtype=np.float32)
    bias = np.asarray(bias, dtype=np.float64)
    I = np.eye(NPOS)
    C_full = _apply_stages(twiddle, I, range(0, 7)).T  # [p, c]
    H = _apply_stages(twiddle, I, range(7, 10)).T      # [p', p]

    cm = np.empty((P, 8, P), np.float32)  # [c', k, s]
    for k in range(8):
        blk = C_full[128 * k : 128 * k + 128, 128 * k : 128 * k + 128]  # [s, c']
        cm[:, k, :] = blk.T
    cm = cm.astype(ml_dtypes.bfloat16)

    pidx = _pidx()
    hg = np.empty((P, 8, P), np.float32)   # [kl, g, kl']
    bias_g = np.empty((P, 8), np.float32)  # [kl', g]
    for g in range(8):
        idx = pidx[:, g]
        hg[:, g, :] = H[np.ix_(idx, idx)].T
        bias_g[:, g] = bias[idx]
    hg = hg.astype(ml_dtypes.bfloat16)

    ident = np.eye(P, dtype=np.float32).astype(ml_dtypes.bfloat16)

    # xt: [ncores, sbt, c', k, b] bf16 (c = 128*k + c')
    xt = np.ascontiguousarray(
        x.reshape(N_CORES, SBT_PER_CORE, 512, NCHUNK, P).transpose(0, 1, 4, 3, 2)
    ).astype(ml_dtypes.bfloat16)
    return xt, cm, hg, bias_g, ident


def _unpack_xs2(core_outs):
    # core out: [sbt=8, p=(k',l')=128, g=8, b=512] -> [4096, 1024] f32
    parts = []
    for o in core_outs:
        arr = np.asarray(o).astype(np.float32)
        # [sbt, k', l', g, b] -> [sbt, b, k', g, l']
        arr = arr.reshape(8, 8, 16, 8, 512).transpose(0, 4, 1, 3, 2)
        parts.append(arr.reshape(4096, 1024))
    return np.concatenate(parts, axis=0)


def _build_xs2(
    repeats: int = 1,
    act_b: int = CFG["act_b"],
    xtp_bufs: int = CFG["xtp_bufs"],
    zint_bufs: int = CFG["zint_bufs"],
    op_bufs: int = CFG["op_bufs"],
    a_act: int = CFG["a_act"],
    t_act: int = CFG["t_act"],
) -> bass.Bass:
    nc = bacc.Bacc()
    f32 = mybir.dt.float32
    bf16 = mybir.dt.bfloat16

    xt = nc.declare_dram_parameter("xt", [SBT_PER_CORE, P, NCHUNK, 512], bf16, isOutput=False)
    cm = nc.declare_dram_parameter("cm", [P, 8, P], bf16, isOutput=False)
    hg = nc.declare_dram_parameter("hg", [P, 8, P], bf16, isOutput=False)
    bias_g = nc.declare_dram_parameter("bias_g", [P, 8], f32, isOutput=False)
    ident = nc.declare_dram_parameter("ident", [P, P], bf16, isOutput=False)
    out = nc.declare_dram_parameter(
        "out", [SBT_PER_CORE, P, 8, 512], bf16, isOutput=True
    )

    with TileContext(nc) as tc:
        with (
            tc.tile_pool(name="const", bufs=1) as cpool,
            tc.tile_pool(name="xtp", bufs=xtp_bufs) as xpool,
            tc.tile_pool(name="zpp", bufs=2) as zpool,
            tc.tile_pool(name="zint", bufs=zint_bufs) as zipool,
            tc.tile_pool(name="outp", bufs=op_bufs) as opool,
            tc.tile_pool(name="psA", bufs=2, space="PSUM") as psA_pool,
            tc.tile_pool(name="psT", bufs=2, space="PSUM") as psT_pool,
            tc.tile_pool(name="psB", bufs=2, space="PSUM") as psB_pool,
        ):
            cm_sb = cpool.tile([P, 8, P], bf16)
            nc.sync.dma_start(out=cm_sb[:], in_=cm[:])
            hg_sb = cpool.tile([P, 8, P], bf16)
            nc.sync.dma_start(out=hg_sb[:], in_=hg[:])
            bias_sb = cpool.tile([P, 8], f32)
            nc.sync.dma_start(out=bias_sb[:], in_=bias_g[:])
            id_sb = cpool.tile([P, P], bf16)
            nc.sync.dma_start(out=id_sb[:], in_=ident[:])

            def do_a(bc, xt_sb, zpp):
                psA = psA_pool.tile([P, 8, P], f32)
                for k in range(8):
                    nc.tensor.matmul(
                        psA[:, k, :],
                        lhsT=xt_sb[:, k, 128 * bc : 128 * (bc + 1)],
                        rhs=cm_sb[:, k, :],
                        start=True, stop=True,
                    )
                eng = nc.scalar.copy if bc < a_act else nc.vector.tensor_copy
                eng(
                    out=zpp[:, bc],
                    in_=psA.rearrange("p k (g l) -> p g k l", g=8, l=16),
                )

            def do_gpair(prev_zpp, j, o_big):
                psT = psT_pool.tile([P, 2, 4, P], bf16)
                for gs in range(2):
                    for bc in range(4):
                        nc.tensor.transpose(
                            psT[:, gs, bc, :],
                            in_=prev_zpp[:, bc, 2 * j + gs].rearrange(
                                "p k l -> p (k l)"
                            ),
                            identity=id_sb[:],
                        )
                zint = zipool.tile([P, 512], f32)
                src = psT.rearrange("p a b c -> p (a b c)").bitcast(f32)
                if j < t_act:
                    nc.scalar.copy(out=zint[:], in_=src)
                else:
                    nc.vector.tensor_copy(out=zint[:], in_=src)
                for gs in range(2):
                    g = 2 * j + gs
                    psB = psB_pool.tile([P, 512], f32)
                    nc.tensor.matmul(
                        psB[:],
                        lhsT=hg_sb[:, g, :],
                        rhs=zint[:, 256 * gs : 256 * (gs + 1)].bitcast(bf16),
                        start=True, stop=True,
                    )
                    if g < act_b:
                        nc.scalar.add(
                            out=o_big[:, g], in_=psB[:], add=bias_sb[:, g : g + 1]
                        )
                    else:
                        nc.vector.tensor_scalar_add(
                            out=o_big[:, g], in0=psB[:],
                            scalar1=bias_sb[:, g : g + 1],
                        )

            for _rep in range(repeats):
                prev = None  # (zpp, o_big, sbt)
                for sbt in range(SBT_PER_CORE + 1):
                    if sbt < SBT_PER_CORE:
                        xt_sb = xpool.tile([P, NCHUNK, 512], bf16)
                        nc.sync.dma_start(out=xt_sb[:], in_=xt[sbt])
                        zpp = zpool.tile([P, 4, 8, 8, 16], bf16)
                        o_big = opool.tile([P, 8, 512], bf16)
                    for bc in range(4):
                        if sbt < SBT_PER_CORE:
                            do_a(bc, xt_sb, zpp)
                        if prev is not None:
                            do_gpair(prev[0], bc, prev[1])
                    if prev is not None:
                        nc.sync.dma_start(
                            out=out[prev[2]].rearrange("p g b -> p (g b)"),
                            in_=prev[1].rearrange("p g b -> p (g b)"),
                        )
                    prev = (zpp, o_big, sbt) if sbt < SBT_PER_CORE else None
    nc.compile()
    return nc


def kernel(x, twiddle, bias, _repeats=1):
    """Harness entry point: full inputs in, full float32 output out."""
    xt, cm, hg, bias_g, ident = _pack_xs(x, twiddle, bias)
    nc = _build_xs2(repeats=_repeats)
    in_maps = [
        {"xt": xt[k], "cm": cm, "hg": hg, "bias_g": bias_g, "ident": ident}
        for k in range(N_CORES)
    ]
    res = run_bass_kernel_spmd(nc, in_maps, list(range(N_CORES)))
    return _unpack_xs2([r["out"] for r in res.results])


# revision 4
# speedup vs baseline: 1.0960x; 1.0960x over previous
"""Butterfly (10-stage, n=1024) on 8 TRN2 cores — 3-phase factorized kernel.

Math: with nstack=1, the 10 butterfly stages compose into a dense W
(out = W x + bias). Split position index p = 128*k + s, s = 16*g + l:
stages 0-6 form a block-diagonal C (8 independent 128x128 blocks, block k
maps c-block k -> s of block k); stages 7-9 mix ONLY k at fixed s (an 8x8
matrix H8[s] per position-within-block s).

Device pipeline per super-tile (sbt = 512 batch rows; core handles 8):
  A: 32 matmuls, x-chunks STATIONARY ([c'=128, b=128] bf16), C-blocks
     moving (N=128) -> z' batch-major [b=128, (k, s)] f32 in PSUM.
     Copy to SBUF bf16 with (k, (g, l)) -> (g, k, l) column permute
     (ACT/DVE split).
  T: 32 PE transposes of z' slices [b=128, (k,l)=128] -> zint [(k,l), b]
     bf16 in PSUM — this performs the partition interleave that any
     2-phase scheme must otherwise pay 2x PE streaming for.
     Copies to SBUF as pure bit-moves (bf16 pairs viewed as f32: half
     the elements).
  B: 8 matmuls (N=512), stationary hg[g] = H8 entries packed per 16-row
     group g; contract over all 128 (k,l) -> out tile [(k',l'), b] f32.
     Bias added during the PSUM->SBUF copy (per-partition scalar);
     output staged bf16 and written with ONE merged 1MB DMA per sbt
     (8KB contiguous per partition line).

PE cost is 12,288 moving-rows/sbt (the structural floor: 3 full passes
over the data; any transpose-free 2-phase split conserves 24,576 —
that 2x tax is exactly what the previous 2-level kernel paid). PE floor
= 98,304 rows/pass @ 2.4GHz = 41.0us. Measured (quiet device): 42.4 and
45.7us/pass vs 86.5-90.7us for the previous kernel (~2x); max rel err
vs f64 reference 6.36e-3 (gate 2e-2). bf16 in/out halves DMA vs f32;
merged 8KB-line descriptors keep DMA far off the critical path.

Per-core DRAM I/O: xt 8MB in, out 8MB out (bf16).
"""

import numpy as np
import ml_dtypes

import concourse.bass as bass
import concourse.bacc as bacc
import concourse.mybir as mybir
from concourse.tile import TileContext
from concourse.bass_utils import run_bass_kernel_spmd

N_CORES = 8
BATCH = 32768
NPOS = 1024
P = 128
SBT_PER_CORE = 8
NCHUNK = 8

# engine-balance / buffering defaults (HW-tuned)
CFG = dict(act_b=4, xtp_bufs=3, zint_bufs=3, op_bufs=3, a_act=2, t_act=3)


def _apply_stages(tw, v, stages):
    """Apply butterfly stages (f64) to rows of v: v |-> B_stages v."""
    b, n = v.shape
    out = v.reshape(b, 1, n)
    tw = np.asarray(tw, dtype=np.float64)
    for idx in stages:
        stride = 1 << idx
        nb = n // (2 * stride)
        t = tw[:, idx].reshape(1, nb, stride, 2, 2).transpose(0, 1, 3, 4, 2)
        o = out.reshape(b, 1, nb, 1, 2, stride)
        out = (t * o).sum(axis=4).reshape(b, 1, n)
    return out.reshape(b, n)


def _compose_w(twiddle):
    """Full composed transform: M[c, p] with out = x @ M (for host checks)."""
    return _apply_stages(twiddle, np.eye(NPOS), range(10))


def _pidx():
    """pidx[kl, g] = position 128*(kl//16) + 16*g + (kl%16)."""
    kl = np.arange(128)
    g = np.arange(8)
    return 128 * (kl[:, None] // 16) + 16 * g[None, :] + (kl[:, None] % 16)


def _pack_xs(x, twiddle, bias):
    x = np.asarray(x, dtype=np.float32)
    bias = np.asarray(bias, dtype=np.float64)
    I = np.eye(NPOS)
    C_full = _apply_stages(twiddle, I, range(0, 7)).T  # [p, c]
    H = _apply_stages(twiddle, I, range(7, 10)).T      # [p', p]

    cm = np.empty((P, 8, P), np.float32)  # [c', k, s]
    for k in range(8):
        blk = C_full[128 * k : 128 * k + 128, 128 * k : 128 * k + 128]  # [s, c']
        cm[:, k, :] = blk.T
    cm = cm.astype(ml_dtypes.bfloat16)

    pidx = _pidx()
    hg = np.empty((P, 8, P), np.float32)   # [kl, g, kl']
    bias_g = np.empty((P, 8), np.float32)  # [kl', g]
    for g in range(8):
        idx = pidx[:, g]
        hg[:, g, :] = H[np.ix_(idx, idx)].T
        bias_g[:, g] = bias[idx]
    hg = hg.astype(ml_dtypes.bfloat16)

    ident = np.eye(P, dtype=np.float32).astype(ml_dtypes.bfloat16)

    # xt: [ncores, sbt, c', k, b] bf16 (c = 128*k + c')
    xt = np.ascontiguousarray(
        x.reshape(N_CORES, SBT_PER_CORE, 512, NCHUNK, P).transpose(0, 1, 4, 3, 2)
    ).astype(ml_dtypes.bfloat16)
    return xt, cm, hg, bias_g, ident


def _unpack_xs2(core_outs):
    # core out: [sbt=8, p=(k',l')=128, g=8, b=512] -> [4096, 1024] f32
    parts = []
    for o in core_outs:
        arr = np.asarray(o).astype(np.float32)
        # [sbt, k', l', g, b] -> [sbt, b, k', g, l']
        arr = arr.reshape(8, 8, 16, 8, 512).transpose(0, 4, 1, 3, 2)
        parts.append(arr.reshape(4096, 1024))
    return np.concatenate(parts, axis=0)


def _build_xs2(
    repeats: int = 1,
    act_b: int = CFG["act_b"],
    xtp_bufs: int = CFG["xtp_bufs"],
    zint_bufs: int = CFG["zint_bufs"],
    op_bufs: int = CFG["op_bufs"],
    a_act: int = CFG["a_act"],
    t_act: int = CFG["t_act"],
    split_out: bool = False,
) -> bass.Bass:
    nc = bacc.Bacc()
    f32 = mybir.dt.float32
    bf16 = mybir.dt.bfloat16

    xt = nc.declare_dram_parameter("xt", [SBT_PER_CORE, P, NCHUNK, 512], bf16, isOutput=False)
    cm = nc.declare_dram_parameter("cm", [P, 8, P], bf16, isOutput=False)
    hg = nc.declare_dram_parameter("hg", [P, 8, P], bf16, isOutput=False)
    bias_g = nc.declare_dram_parameter("bias_g", [P, 8], f32, isOutput=False)
    ident = nc.declare_dram_parameter("ident", [P, P], bf16, isOutput=False)
    out = nc.declare_dram_parameter(
        "out", [SBT_PER_CORE, P, 8, 512], bf16, isOutput=True
    )

    with TileContext(nc) as tc:
        with (
            tc.tile_pool(name="const", bufs=1) as cpool,
            tc.tile_pool(name="xtp", bufs=xtp_bufs) as xpool,
            tc.tile_pool(name="zpp", bufs=2) as zpool,
            tc.tile_pool(name="zint", bufs=zint_bufs) as zipool,
            tc.tile_pool(name="outp", bufs=op_bufs) as opool,
            tc.tile_pool(name="psA", bufs=2, space="PSUM") as psA_pool,
            tc.tile_pool(name="psT", bufs=2, space="PSUM") as psT_pool,
            tc.tile_pool(name="psB", bufs=2, space="PSUM") as psB_pool,
        ):
            cm_sb = cpool.tile([P, 8, P], bf16)
            nc.sync.dma_start(out=cm_sb[:], in_=cm[:])
            hg_sb = cpool.tile([P, 8, P], bf16)
            nc.sync.dma_start(out=hg_sb[:], in_=hg[:])
            bias_sb = cpool.tile([P, 8], f32)
            nc.sync.dma_start(out=bias_sb[:], in_=bias_g[:])
            id_sb = cpool.tile([P, P], bf16)
            nc.sync.dma_start(out=id_sb[:], in_=ident[:])

            def do_a(bc, xt_sb, zpp):
                psA = psA_pool.tile([P, 8, P], f32)
                for k in range(8):
                    nc.tensor.matmul(
                        psA[:, k, :],
                        lhsT=xt_sb[:, k, 128 * bc : 128 * (bc + 1)],
                        rhs=cm_sb[:, k, :],
                        start=True, stop=True,
                    )
                eng = nc.scalar.copy if bc < a_act else nc.vector.tensor_copy
                eng(
                    out=zpp[:, bc],
                    in_=psA.rearrange("p k (g l) -> p g k l", g=8, l=16),
                )

            def do_gpair(prev_zpp, j, o_big):
                psT = psT_pool.tile([P, 2, 4, P], bf16)
                for gs in range(2):
                    for bc in range(4):
                        nc.tensor.transpose(
                            psT[:, gs, bc, :],
                            in_=prev_zpp[:, bc, 2 * j + gs].rearrange(
                                "p k l -> p (k l)"
                            ),
                            identity=id_sb[:],
                        )
                zint = zipool.tile([P, 512], f32)
                src = psT.rearrange("p a b c -> p (a b c)").bitcast(f32)
                if j < t_act:
                    nc.scalar.copy(out=zint[:], in_=src)
                else:
                    nc.vector.tensor_copy(out=zint[:], in_=src)
                for gs in range(2):
                    g = 2 * j + gs
                    psB = psB_pool.tile([P, 512], f32)
                    nc.tensor.matmul(
                        psB[:],
                        lhsT=hg_sb[:, g, :],
                        rhs=zint[:, 256 * gs : 256 * (gs + 1)].bitcast(bf16),
                        start=True, stop=True,
                    )
                    if g < act_b:
                        nc.scalar.add(
                            out=o_big[:, g], in_=psB[:], add=bias_sb[:, g : g + 1]
                        )
                    else:
                        nc.vector.tensor_scalar_add(
                            out=o_big[:, g], in0=psB[:],
                            scalar1=bias_sb[:, g : g + 1],
                        )

            for _rep in range(repeats):
                prev = None  # (zpp, o_big, sbt)
                for sbt in range(SBT_PER_CORE + 1):
                    if sbt < SBT_PER_CORE:
                        xt_sb = xpool.tile([P, NCHUNK, 512], bf16)
                        nc.sync.dma_start(out=xt_sb[:], in_=xt[sbt])
                        zpp = zpool.tile([P, 4, 8, 8, 16], bf16)
                        o_big = opool.tile([P, 8, 512], bf16)
                    for bc in range(4):
                        if sbt < SBT_PER_CORE:
                            do_a(bc, xt_sb, zpp)
                        if prev is not None:
                            do_gpair(prev[0], bc, prev[1])
                            if split_out and bc == 1:
                                nc.sync.dma_start(
                                    out=out[prev[2], :, 0:4].rearrange(
                                        "p g b -> p (g b)"
                                    ),
                                    in_=prev[1][:, 0:4].rearrange("p g b -> p (g b)"),
                                )
                    if prev is not None:
                        if split_out:
                            nc.sync.dma_start(
                                out=out[prev[2], :, 4:8].rearrange("p g b -> p (g b)"),
                                in_=prev[1][:, 4:8].rearrange("p g b -> p (g b)"),
                            )
                        else:
                            nc.sync.dma_start(
                                out=out[prev[2]].rearrange("p g b -> p (g b)"),
                                in_=prev[1].rearrange("p g b -> p (g b)"),
                            )
                    prev = (zpp, o_big, sbt) if sbt < SBT_PER_CORE else None
    nc.compile()
    return nc


def kernel(x, twiddle, bias, _repeats=1):
    """Harness entry point: full inputs in, full float32 output out."""
    xt, cm, hg, bias_g, ident = _pack_xs(x, twiddle, bias)
    nc = _build_xs2(repeats=_repeats)
    in_maps = [
        {"xt": xt[k], "cm": cm, "hg": hg, "bias_g": bias_g, "ident": ident}
        for k in range(N_CORES)
    ]
    res = run_bass_kernel_spmd(nc, in_maps, list(range(N_CORES)))
    return _unpack_xs2([r["out"] for r in res.results])
